# revision 4
# baseline (speedup 1.0000x reference)
"""Trainium2 Bass kernel for EnhancedPrototypeMemory (scatter_memory).

Strategy (8 NeuronCores, data-parallel over N):
  - Each core streams its N/8 = 16384 feature rows once from HBM (memory-bound).
  - Per 128-row tile: row sum-of-squares via ACT Square+accum, sqrt/reciprocal
    for the L2 norm, then a one-hot matrix scaled by 1/||f|| is built on DVE
    (fused is_equal+mult tensor_scalar against an iota constant).
  - PE matmul accumulates onehot_scaled.T @ [features | norm] into a [91,257]
    PSUM tile: columns 0..255 are the normalized per-class sums, column 256 is
    the per-class count (rnorm * norm = 1 per member row).
  - The [91,257] partials are AllReduce'd across the 8 cores (tiny, ~93 KB),
    then every core applies the identical EMA epilogue on [91,256] tiles.
Host only shards/reassembles and converts bool/int masks to f32 and back.
"""

import numpy as np

import concourse.bacc as bacc
import concourse.mybir as mybir
import concourse.tile as tile
from concourse.bass_utils import run_bass_kernel_spmd

# Problem constants (hardcoded per contract; kernel.py must be self-contained).
N_CORES = 8
C = 91
D = 256
N_TOTAL = 131072
N_SHARD = N_TOTAL // N_CORES  # 16384
P = 128
T = N_SHARD // P  # 128 tiles per core
MOMENTUM = 0.999
WARMUP_STEPS = 200
BASE_MOM = 0.99
SHADOW_M = min(MOMENTUM + 0.0009, 0.9999)  # 0.9999

# Tunables.
G = 4          # feature tiles per DMA group
F_BUFS = 4     # feature-tile double buffering
MATMUL_F32R = False  # use PE fast-fp32 mode (1 cyc/row at N>=256 vs 4 for fp32)

FP = mybir.dt.float32
AF = mybir.ActivationFunctionType
OP = mybir.AluOpType

_cache: dict = {}


def _build(progress: float):
    nc = bacc.Bacc("TRN2", target_bir_lowering=False, debug=False,
                   num_devices=N_CORES)

    feat = nc.dram_tensor("feat", [N_SHARD, D], FP, kind="ExternalInput")
    labels_t = nc.dram_tensor("labels_t", [P, T], FP, kind="ExternalInput")
    proto_in = nc.dram_tensor("proto_in", [C, D], FP, kind="ExternalInput")
    init_in = nc.dram_tensor("init_in", [C, 1], FP, kind="ExternalInput")
    var_in = nc.dram_tensor("var_in", [C, 1], FP, kind="ExternalInput")
    shadow_in = nc.dram_tensor("shadow_in", [C, D], FP, kind="ExternalInput")
    sinit_in = nc.dram_tensor("sinit_in", [C, 1], FP, kind="ExternalInput")
    count_in = nc.dram_tensor("count_in", [C, 1], FP, kind="ExternalInput")

    newp_out = nc.dram_tensor("new_proto", [C, D], FP, kind="ExternalOutput")
    newv_out = nc.dram_tensor("new_var", [C, 1], FP, kind="ExternalOutput")
    news_out = nc.dram_tensor("new_shadow", [C, D], FP, kind="ExternalOutput")
    newi_out = nc.dram_tensor("new_init", [C, 1], FP, kind="ExternalOutput")
    newsi_out = nc.dram_tensor("new_sinit", [C, 1], FP, kind="ExternalOutput")
    newc_out = nc.dram_tensor("new_count", [C, 1], FP, kind="ExternalOutput")

    iota_const = nc.inline_tensor(
        np.tile(np.arange(C, dtype=np.float32), (P, 1)), name="iota_const")

    k_mom = float(np.float32(MOMENTUM - BASE_MOM))  # scaled at runtime by progress

    with tile.TileContext(nc) as tc:
        with (
            tc.tile_pool(name="const", bufs=1) as cpool,
            tc.tile_pool(name="feat", bufs=F_BUFS) as fpool,
            tc.tile_pool(name="oh", bufs=8) as ohpool,
            tc.tile_pool(name="stats", bufs=4) as spool,
            tc.tile_pool(name="scratch", bufs=1) as scrpool,
            tc.tile_pool(name="psum", bufs=1, space="PSUM") as pspool,
            tc.tile_pool(name="epi", bufs=1) as epool,
            tc.tile_pool(name="dram", bufs=1, space="DRAM") as dram,
        ):
            # One-time constants / small inputs.
            iota_sb = cpool.tile([P, C], FP, tag="iota")
            nc.sync.dma_start(iota_sb[:], iota_const[:, :])
            labels_sb = cpool.tile([P, T], FP, tag="labels")
            nc.sync.dma_start(labels_sb[:], labels_t[:, :])

            # Epilogue inputs (loaded early; overlap with main loop).
            proto_sb = epool.tile([C, D], FP, tag="proto")
            nc.sync.dma_start(proto_sb[:], proto_in[:, :])
            shadow_sb = epool.tile([C, D], FP, tag="shadow")
            nc.sync.dma_start(shadow_sb[:], shadow_in[:, :])
            init_sb = epool.tile([C, 1], FP, tag="init")
            nc.sync.dma_start(init_sb[:], init_in[:, :])
            var_sb = epool.tile([C, 1], FP, tag="var")
            nc.sync.dma_start(var_sb[:], var_in[:, :])
            sinit_sb = epool.tile([C, 1], FP, tag="sinit")
            nc.sync.dma_start(sinit_sb[:], sinit_in[:, :])
            count_sb = epool.tile([C, 1], FP, tag="count")
            nc.sync.dma_start(count_sb[:], count_in[:, :])

            sq_scratch = scrpool.tile([P, D], FP, tag="sq_scratch")

            psum = pspool.tile([C, D + 1], FP, tag="acc")

            feat_r = feat[:, :].rearrange("(t p) d -> p t d", p=P)

            for g0 in range(0, T, G):
                ftile = fpool.tile([P, G, D + 1], FP, tag="ftile")
                nc.sync.dma_start(ftile[:, :, 0:D], feat_r[:, g0:g0 + G, :])

                sumsq = spool.tile([P, G], FP, tag="sumsq")
                for g in range(G):
                    nc.scalar.activation(
                        sq_scratch[:], ftile[:, g, 0:D], AF.Square,
                        accum_out=sumsq[:, g:g + 1])
                norms = spool.tile([P, G], FP, tag="norms")
                nc.scalar.activation(norms[:], sumsq[:], AF.Sqrt)
                # norm goes into rhs column 256 so the matmul emits exact counts
                nc.vector.tensor_copy(ftile[:, :, D:D + 1], norms[:])
                rnorm = spool.tile([P, G], FP, tag="rnorm")
                nc.vector.reciprocal(rnorm[:], norms[:])

                for g in range(G):
                    t = g0 + g
                    oh = ohpool.tile([P, C], FP, tag="oh")
                    nc.vector.tensor_scalar(
                        oh[:], iota_sb[:],
                        labels_sb[:, t:t + 1], rnorm[:, g:g + 1],
                        op0=OP.is_equal, op1=OP.mult)
                    lhsT = oh[:]
                    rhs = ftile[:, g, :]
                    if MATMUL_F32R:
                        lhsT = lhsT.bitcast(mybir.dt.float32r)
                        rhs = rhs.bitcast(mybir.dt.float32r)
                    nc.tensor.matmul(
                        psum[:], lhsT, rhs,
                        start=(t == 0), stop=(t == T - 1))

            # ---- cross-core reduce of [C, 257] partials ----
            partial = epool.tile([C, D + 1], FP, tag="partial")
            nc.scalar.copy(partial[:], psum[:])
            cc_in = dram.tile([C, D + 1], FP, tag="cc_in")
            cc_out = dram.tile([C, D + 1], FP, tag="cc_out")
            nc.gpsimd.dma_start(cc_in[:], partial[:])
            nc.gpsimd.collective_compute(
                "AllReduce", OP.add,
                replica_groups=[list(range(N_CORES))],
                ins=[cc_in[:].opt()], outs=[cc_out[:].opt()])
            total = epool.tile([C, D + 1], FP, tag="total")
            nc.gpsimd.dma_start(total[:], cc_out[:])

            # ---- epilogue (identical on every core) ----
            sums = total[:, 0:D]
            counts = total[:, D:D + 1]

            def etile(tag, shape=(C, 1)):
                return epool.tile(list(shape), FP, tag=tag, name=tag)

            present = etile("present")
            nc.vector.tensor_scalar(present[:], counts, 0.0, None, op0=OP.is_gt)
            cntc = etile("cntc")
            nc.vector.tensor_scalar(cntc[:], counts, 1.0, None, op0=OP.max)
            inv = etile("inv")
            nc.vector.reciprocal(inv[:], cntc[:])
            cls = etile("cls", (C, D))
            nc.vector.tensor_scalar(cls[:], sums, inv[:], None, op0=OP.mult)

            # alpha = present * (1 - init*mom);  mom = BASE + k*progress*exp(-var)
            e = etile("e")
            nc.scalar.activation(e[:], var_sb[:], AF.Exp, scale=-1.0)
            momt = etile("momt")  # init * k * progress * exp(-var)
            nc.vector.tensor_scalar(momt[:], e[:], float(k_mom * progress),
                                    init_sb[:], op0=OP.mult, op1=OP.mult)
            a1 = etile("a1")  # 1 - BASE*init
            nc.vector.tensor_scalar(a1[:], init_sb[:], float(-BASE_MOM), 1.0,
                                    op0=OP.mult, op1=OP.add)
            a2 = etile("a2")
            nc.vector.tensor_tensor(a2[:], a1[:], momt[:], op=OP.subtract)
            alpha = etile("alpha")
            nc.vector.tensor_tensor(alpha[:], a2[:], present[:], op=OP.mult)

            d = etile("d", (C, D))  # cls_feat - old  (reused for upd_mag)
            nc.vector.tensor_tensor(d[:], cls[:], proto_sb[:], op=OP.subtract)
            t1 = etile("t1", (C, D))
            nc.vector.tensor_scalar(t1[:], d[:], alpha[:], None, op0=OP.mult)
            newp = etile("newp", (C, D))
            nc.vector.tensor_tensor(newp[:], proto_sb[:], t1[:], op=OP.add)

            sq91 = etile("sq91", (C, D))
            ss = etile("ss")
            nc.scalar.activation(sq91[:], d[:], AF.Square, accum_out=ss[:])
            mag = etile("mag")
            nc.scalar.activation(mag[:], ss[:], AF.Sqrt)

            w = etile("w")  # 0.01 * present * init
            nc.vector.tensor_scalar(w[:], present[:], init_sb[:], 0.01,
                                    op0=OP.mult, op1=OP.mult)
            g2 = etile("g2")
            nc.vector.tensor_tensor(g2[:], mag[:], var_sb[:], op=OP.subtract)
            wg = etile("wg")
            nc.vector.tensor_tensor(wg[:], g2[:], w[:], op=OP.mult)
            newv = etile("newv")
            nc.vector.tensor_tensor(newv[:], var_sb[:], wg[:], op=OP.add)

            b1 = etile("b1")  # 1 - SHADOW_M*sinit
            nc.vector.tensor_scalar(b1[:], sinit_sb[:], float(-SHADOW_M), 1.0,
                                    op0=OP.mult, op1=OP.add)
            beta = etile("beta")
            nc.vector.tensor_tensor(beta[:], b1[:], present[:], op=OP.mult)
            d2 = etile("d2", (C, D))
            nc.vector.tensor_tensor(d2[:], newp[:], shadow_sb[:], op=OP.subtract)
            t2 = etile("t2", (C, D))
            nc.vector.tensor_scalar(t2[:], d2[:], beta[:], None, op0=OP.mult)
            news = etile("news", (C, D))
            nc.vector.tensor_tensor(news[:], shadow_sb[:], t2[:], op=OP.add)

            newi = etile("newi")
            nc.vector.tensor_tensor(newi[:], init_sb[:], present[:], op=OP.max)
            newsi = etile("newsi")
            nc.vector.tensor_tensor(newsi[:], sinit_sb[:], present[:], op=OP.max)
            newc = etile("newc")
            nc.vector.tensor_tensor(newc[:], count_sb[:], present[:], op=OP.add)

            nc.sync.dma_start(newp_out[:, :], newp[:])
            nc.sync.dma_start(newv_out[:, :], newv[:])
            nc.sync.dma_start(news_out[:, :], news[:])
            nc.sync.dma_start(newi_out[:, :], newi[:])
            nc.sync.dma_start(newsi_out[:, :], newsi[:])
            nc.sync.dma_start(newc_out[:, :], newc[:])

    nc.finalize()
    return nc


def kernel(features, labels, prototypes, proto_initialized, proto_variance,
           shadow_prototypes, shadow_initialized, proto_update_count, step):
    features = np.ascontiguousarray(np.asarray(features, dtype=np.float32))
    labels = np.asarray(labels)
    prototypes = np.asarray(prototypes, dtype=np.float32)
    proto_initialized = np.asarray(proto_initialized)
    proto_variance = np.asarray(proto_variance, dtype=np.float32)
    shadow_prototypes = np.asarray(shadow_prototypes, dtype=np.float32)
    shadow_initialized = np.asarray(shadow_initialized)
    proto_update_count = np.asarray(proto_update_count)
    count_dtype = proto_update_count.dtype

    progress = min(1.0, float(step) / max(1, WARMUP_STEPS * 10))

    key = (features.shape, float(progress))
    nc = _cache.get(key)
    if nc is None:
        nc = _build(progress)
        _cache[key] = nc

    proto2 = prototypes.reshape(C, D)
    init_f = proto_initialized.astype(np.float32).reshape(C, 1)
    var2 = proto_variance.reshape(C, 1)
    shadow2 = shadow_prototypes.reshape(C, D)
    sinit_f = shadow_initialized.astype(np.float32).reshape(C, 1)
    count_f = proto_update_count.astype(np.float32).reshape(C, 1)

    in_maps = []
    for i in range(N_CORES):
        sl = slice(i * N_SHARD, (i + 1) * N_SHARD)
        lab = labels[sl].astype(np.float32).reshape(T, P).T.copy()
        in_maps.append({
            "feat": features[sl],
            "labels_t": lab,
            "proto_in": proto2,
            "init_in": init_f,
            "var_in": var2,
            "shadow_in": shadow2,
            "sinit_in": sinit_f,
            "count_in": count_f,
        })

    res = run_bass_kernel_spmd(nc, in_maps, core_ids=list(range(N_CORES)))
    r0 = res.results[0]

    new_proto = r0["new_proto"].reshape(C, D)
    new_var = r0["new_var"].reshape(C)
    new_shadow = r0["new_shadow"].reshape(C, D)
    new_init = r0["new_init"].reshape(C) > 0.5
    new_sinit = r0["new_sinit"].reshape(C) > 0.5
    new_count = np.rint(r0["new_count"].reshape(C)).astype(count_dtype)
    return (new_proto, new_var, new_shadow, new_init, new_sinit, new_count)


# revision 6
# speedup vs baseline: 1.0023x; 1.0023x over previous
"""Trainium2 Bass kernel for EnhancedPrototypeMemory (scatter_memory).

Strategy (8 NeuronCores, data-parallel over N):
  - Each core streams its N/8 = 16384 feature rows once from HBM (memory-bound)
    with an on-the-fly f32->fp16 cast in the SWDGE DMA (full 128 MiB still read
    from HBM; SBUF tiles and PE operands are fp16).
  - Per 128-row tile: row sum-of-squares via ACT Square+accum (fp32 accum),
    sqrt/reciprocal for the L2 norm, then a one-hot matrix scaled by 1/||f||
    is built on DVE (fused is_equal+mult tensor_scalar against an iota const).
  - PE matmul (fp16 x fp16 -> fp32 PSUM) accumulates onehot_scaled.T @
    [features | norm] into a [91,257] PSUM tile: columns 0..255 are the
    normalized per-class sums, column 256 is the per-class count
    (rnorm * norm = 1 per member row); counts are re-rounded to exact
    integers in the epilogue.
  - The [91,257] partials are AllReduce'd across the 8 cores (tiny, ~93 KB),
    then every core applies the identical EMA epilogue on [91,256] tiles.
Host only shards/reassembles and converts bool/int masks to f32 and back.

fp16 error analysis: each row's unit vector picks up ~2^-12 relative noise;
class means average ~1440 rows so the graded absmax-relative error lands at
~1e-5, while counts are re-rounded to exact integers.
"""

import numpy as np

import concourse.bacc as bacc
import concourse.mybir as mybir
import concourse.tile as tile
from concourse.bass_utils import run_bass_kernel_spmd

# Problem constants (hardcoded per contract; kernel.py must be self-contained).
N_CORES = 8
C = 91
D = 256
N_TOTAL = 131072
N_SHARD = N_TOTAL // N_CORES  # 16384
P = 128
T = N_SHARD // P  # 128 tiles per core
MOMENTUM = 0.999
WARMUP_STEPS = 200
BASE_MOM = 0.99
SHADOW_M = min(MOMENTUM + 0.0009, 0.9999)  # 0.9999

# Tunables.
G = 4          # feature tiles per DMA group
F_BUFS = 4     # feature-tile double buffering

FP = mybir.dt.float32
FH = mybir.dt.float16
AF = mybir.ActivationFunctionType
OP = mybir.AluOpType

_cache: dict = {}


def _build(progress: float):
    nc = bacc.Bacc("TRN2", target_bir_lowering=False, debug=False,
                   num_devices=N_CORES)

    feat = nc.dram_tensor("feat", [N_SHARD, D], FP, kind="ExternalInput")
    labels_t = nc.dram_tensor("labels_t", [P, T], FP, kind="ExternalInput")
    proto_in = nc.dram_tensor("proto_in", [C, D], FP, kind="ExternalInput")
    init_in = nc.dram_tensor("init_in", [C, 1], FP, kind="ExternalInput")
    var_in = nc.dram_tensor("var_in", [C, 1], FP, kind="ExternalInput")
    shadow_in = nc.dram_tensor("shadow_in", [C, D], FP, kind="ExternalInput")
    sinit_in = nc.dram_tensor("sinit_in", [C, 1], FP, kind="ExternalInput")
    count_in = nc.dram_tensor("count_in", [C, 1], FP, kind="ExternalInput")

    newp_out = nc.dram_tensor("new_proto", [C, D], FP, kind="ExternalOutput")
    newv_out = nc.dram_tensor("new_var", [C, 1], FP, kind="ExternalOutput")
    news_out = nc.dram_tensor("new_shadow", [C, D], FP, kind="ExternalOutput")
    newi_out = nc.dram_tensor("new_init", [C, 1], FP, kind="ExternalOutput")
    newsi_out = nc.dram_tensor("new_sinit", [C, 1], FP, kind="ExternalOutput")
    newc_out = nc.dram_tensor("new_count", [C, 1], FP, kind="ExternalOutput")

    iota_const = nc.inline_tensor(
        np.tile(np.arange(C, dtype=np.float32), (P, 1)), name="iota_const")

    k_mom = float(np.float32(MOMENTUM - BASE_MOM))

    with tile.TileContext(nc) as tc:
        with (
            tc.tile_pool(name="const", bufs=1) as cpool,
            tc.tile_pool(name="feat", bufs=F_BUFS) as fpool,
            tc.tile_pool(name="oh", bufs=8) as ohpool,
            tc.tile_pool(name="stats", bufs=4) as spool,
            tc.tile_pool(name="scratch", bufs=1) as scrpool,
            tc.tile_pool(name="psum", bufs=1, space="PSUM") as pspool,
            tc.tile_pool(name="epi", bufs=1) as epool,
            tc.tile_pool(name="dram", bufs=1, space="DRAM") as dram,
        ):
            # One-time constants / small inputs.
            iota_sb = cpool.tile([P, C], FP, tag="iota")
            nc.sync.dma_start(iota_sb[:], iota_const[:, :])
            labels_sb = cpool.tile([P, T], FP, tag="labels")
            nc.sync.dma_start(labels_sb[:], labels_t[:, :])

            # Epilogue inputs (loaded early; overlap with main loop).
            proto_sb = epool.tile([C, D], FP, tag="proto")
            nc.sync.dma_start(proto_sb[:], proto_in[:, :])
            shadow_sb = epool.tile([C, D], FP, tag="shadow")
            nc.sync.dma_start(shadow_sb[:], shadow_in[:, :])
            init_sb = epool.tile([C, 1], FP, tag="init")
            nc.sync.dma_start(init_sb[:], init_in[:, :])
            var_sb = epool.tile([C, 1], FP, tag="var")
            nc.sync.dma_start(var_sb[:], var_in[:, :])
            sinit_sb = epool.tile([C, 1], FP, tag="sinit")
            nc.sync.dma_start(sinit_sb[:], sinit_in[:, :])
            count_sb = epool.tile([C, 1], FP, tag="count")
            nc.sync.dma_start(count_sb[:], count_in[:, :])

            sq_scratch = scrpool.tile([P, D], FH, tag="sq_scratch")

            psum = pspool.tile([C, D + 1], FP, tag="acc")

            feat_r = feat[:, :].rearrange("(t p) d -> p t d", p=P)

            for g0 in range(0, T, G):
                # SWDGE cast-DMA: f32 in HBM -> fp16 in SBUF
                ftile = fpool.tile([P, G, D + 1], FH, tag="ftile")
                nc.gpsimd.dma_start(ftile[:, :, 0:D], feat_r[:, g0:g0 + G, :])

                sumsq = spool.tile([P, G], FP, tag="sumsq")
                for g in range(G):
                    nc.scalar.activation(
                        sq_scratch[:], ftile[:, g, 0:D], AF.Square,
                        accum_out=sumsq[:, g:g + 1])
                norms = spool.tile([P, G], FP, tag="norms")
                nc.scalar.activation(norms[:], sumsq[:], AF.Sqrt)
                # norm goes into rhs column 256 so the matmul emits counts
                nc.vector.tensor_copy(ftile[:, :, D:D + 1], norms[:])
                rnorm = spool.tile([P, G], FP, tag="rnorm")
                nc.vector.reciprocal(rnorm[:], norms[:])

                for g in range(G):
                    t = g0 + g
                    oh = ohpool.tile([P, C], FH, tag="oh")
                    nc.vector.tensor_scalar(
                        oh[:], iota_sb[:],
                        labels_sb[:, t:t + 1], rnorm[:, g:g + 1],
                        op0=OP.is_equal, op1=OP.mult)
                    nc.tensor.matmul(
                        psum[:], oh[:], ftile[:, g, :],
                        start=(t == 0), stop=(t == T - 1))

            # ---- cross-core reduce of [C, 257] partials ----
            partial = epool.tile([C, D + 1], FP, tag="partial")
            nc.scalar.copy(partial[:], psum[:])
            cc_in = dram.tile([C, D + 1], FP, tag="cc_in")
            cc_out = dram.tile([C, D + 1], FP, tag="cc_out")
            nc.sync.dma_start(cc_in[:], partial[:])
            nc.gpsimd.collective_compute(
                "AllReduce", OP.add,
                replica_groups=[list(range(N_CORES))],
                ins=[cc_in[:].opt()], outs=[cc_out[:].opt()])
            total = epool.tile([C, D + 1], FP, tag="total")
            nc.sync.dma_start(total[:], cc_out[:])

            # ---- epilogue (identical on every core) ----
            sums = total[:, 0:D]
            counts_raw = total[:, D:D + 1]

            def etile(tag, shape=(C, 1)):
                return epool.tile(list(shape), FP, tag=tag, name=tag)

            # counts came through fp16 products (rnorm*norm = 1 +- 2^-11 per
            # member); round back to the exact integer via the 2^23 trick
            # (adding 2^23 forces fp32 mantissa alignment -> round-to-int).
            counts = etile("counts")
            nc.vector.tensor_scalar(counts[:], counts_raw, float(2 ** 23),
                                    float(-(2 ** 23)), op0=OP.add, op1=OP.add)

            present = etile("present")
            nc.vector.tensor_scalar(present[:], counts[:], 0.0, None, op0=OP.is_gt)
            cntc = etile("cntc")
            nc.vector.tensor_scalar(cntc[:], counts[:], 1.0, None, op0=OP.max)
            inv = etile("inv")
            nc.vector.reciprocal(inv[:], cntc[:])
            cls = etile("cls", (C, D))
            nc.vector.tensor_scalar(cls[:], sums, inv[:], None, op0=OP.mult)

            # alpha = present * (1 - init*mom);  mom = BASE + k*progress*exp(-var)
            e = etile("e")
            nc.scalar.activation(e[:], var_sb[:], AF.Exp, scale=-1.0)
            momt = etile("momt")
            nc.vector.tensor_scalar(momt[:], e[:], float(k_mom * progress),
                                    init_sb[:], op0=OP.mult, op1=OP.mult)
            a1 = etile("a1")
            nc.vector.tensor_scalar(a1[:], init_sb[:], float(-BASE_MOM), 1.0,
                                    op0=OP.mult, op1=OP.add)
            a2 = etile("a2")
            nc.vector.tensor_tensor(a2[:], a1[:], momt[:], op=OP.subtract)
            alpha = etile("alpha")
            nc.vector.tensor_tensor(alpha[:], a2[:], present[:], op=OP.mult)

            d = etile("d", (C, D))  # cls_feat - old  (reused for upd_mag)
            nc.vector.tensor_tensor(d[:], cls[:], proto_sb[:], op=OP.subtract)
            t1 = etile("t1", (C, D))
            nc.vector.tensor_scalar(t1[:], d[:], alpha[:], None, op0=OP.mult)
            newp = etile("newp", (C, D))
            nc.vector.tensor_tensor(newp[:], proto_sb[:], t1[:], op=OP.add)

            sq91 = etile("sq91", (C, D))
            ss = etile("ss")
            nc.scalar.activation(sq91[:], d[:], AF.Square, accum_out=ss[:])
            mag = etile("mag")
            nc.scalar.activation(mag[:], ss[:], AF.Sqrt)

            w = etile("w")
            nc.vector.tensor_scalar(w[:], present[:], init_sb[:], 0.01,
                                    op0=OP.mult, op1=OP.mult)
            g2 = etile("g2")
            nc.vector.tensor_tensor(g2[:], mag[:], var_sb[:], op=OP.subtract)
            wg = etile("wg")
            nc.vector.tensor_tensor(wg[:], g2[:], w[:], op=OP.mult)
            newv = etile("newv")
            nc.vector.tensor_tensor(newv[:], var_sb[:], wg[:], op=OP.add)

            b1 = etile("b1")
            nc.vector.tensor_scalar(b1[:], sinit_sb[:], float(-SHADOW_M), 1.0,
                                    op0=OP.mult, op1=OP.add)
            beta = etile("beta")
            nc.vector.tensor_tensor(beta[:], b1[:], present[:], op=OP.mult)
            d2 = etile("d2", (C, D))
            nc.vector.tensor_tensor(d2[:], newp[:], shadow_sb[:], op=OP.subtract)
            t2 = etile("t2", (C, D))
            nc.vector.tensor_scalar(t2[:], d2[:], beta[:], None, op0=OP.mult)
            news = etile("news", (C, D))
            nc.vector.tensor_tensor(news[:], shadow_sb[:], t2[:], op=OP.add)

            newi = etile("newi")
            nc.vector.tensor_tensor(newi[:], init_sb[:], present[:], op=OP.max)
            newsi = etile("newsi")
            nc.vector.tensor_tensor(newsi[:], sinit_sb[:], present[:], op=OP.max)
            newc = etile("newc")
            nc.vector.tensor_tensor(newc[:], count_sb[:], present[:], op=OP.add)

            nc.sync.dma_start(newp_out[:, :], newp[:])
            nc.sync.dma_start(newv_out[:, :], newv[:])
            nc.sync.dma_start(news_out[:, :], news[:])
            nc.sync.dma_start(newi_out[:, :], newi[:])
            nc.sync.dma_start(newsi_out[:, :], newsi[:])
            nc.sync.dma_start(newc_out[:, :], newc[:])

    nc.finalize()
    return nc


def kernel(features, labels, prototypes, proto_initialized, proto_variance,
           shadow_prototypes, shadow_initialized, proto_update_count, step):
    features = np.ascontiguousarray(np.asarray(features, dtype=np.float32))
    labels = np.asarray(labels)
    prototypes = np.asarray(prototypes, dtype=np.float32)
    proto_initialized = np.asarray(proto_initialized)
    proto_variance = np.asarray(proto_variance, dtype=np.float32)
    shadow_prototypes = np.asarray(shadow_prototypes, dtype=np.float32)
    shadow_initialized = np.asarray(shadow_initialized)
    proto_update_count = np.asarray(proto_update_count)
    count_dtype = proto_update_count.dtype

    progress = min(1.0, float(step) / max(1, WARMUP_STEPS * 10))

    key = (features.shape, float(progress))
    nc = _cache.get(key)
    if nc is None:
        nc = _build(progress)
        _cache[key] = nc

    proto2 = prototypes.reshape(C, D)
    init_f = proto_initialized.astype(np.float32).reshape(C, 1)
    var2 = proto_variance.reshape(C, 1)
    shadow2 = shadow_prototypes.reshape(C, D)
    sinit_f = shadow_initialized.astype(np.float32).reshape(C, 1)
    count_f = proto_update_count.astype(np.float32).reshape(C, 1)

    in_maps = []
    for i in range(N_CORES):
        sl = slice(i * N_SHARD, (i + 1) * N_SHARD)
        lab = labels[sl].astype(np.float32).reshape(T, P).T.copy()
        in_maps.append({
            "feat": features[sl],
            "labels_t": lab,
            "proto_in": proto2,
            "init_in": init_f,
            "var_in": var2,
            "shadow_in": shadow2,
            "sinit_in": sinit_f,
            "count_in": count_f,
        })

    res = run_bass_kernel_spmd(nc, in_maps, core_ids=list(range(N_CORES)))
    r0 = res.results[0]

    new_proto = r0["new_proto"].reshape(C, D)
    new_var = r0["new_var"].reshape(C)
    new_shadow = r0["new_shadow"].reshape(C, D)
    new_init = r0["new_init"].reshape(C) > 0.5
    new_sinit = r0["new_sinit"].reshape(C) > 0.5
    new_count = np.rint(r0["new_count"].reshape(C)).astype(count_dtype)
    return (new_proto, new_var, new_shadow, new_init, new_sinit, new_count)


# revision 7
# speedup vs baseline: 1.2201x; 1.2173x over previous
"""Trainium2 Bass kernel for EnhancedPrototypeMemory (scatter_memory).

Strategy (8 NeuronCores, data-parallel over N):
  - Each core streams its N/8 = 16384 feature rows once from HBM with an
    on-the-fly f32->fp16 cast in the SWDGE DMA (full 128 MiB still read from
    HBM; SBUF tiles and PE operands are fp16).
  - Per 128-row tile: row sum-of-squares split across engines (a fraction of
    tiles uses ACT Square+accumulate, the rest uses a batched ACT Square into
    scratch + one batched DVE reduce) -> sqrt -> reciprocal; a one-hot matrix
    scaled by 1/||f|| is built on DVE (fused is_equal+mult tensor_scalar
    against an iota constant).
  - PE matmul (fp16 -> fp32 PSUM) accumulates onehot_scaled.T @
    [features | norm] into a [91,257] PSUM tile: cols 0..255 = normalized
    per-class sums, col 256 = per-class count (rnorm*norm = 1 per member);
    counts are re-rounded to exact integers in the epilogue.
  - The [91,257] partials are AllReduce'd across the 8 cores, then every
    core applies the identical EMA epilogue on [91,256] tiles.
  - All six epilogue state inputs are packed into ONE [91,516] input DMA and
    all six outputs into ONE [91,516] output DMA (single completion receipt).
Host only shards/packs/unpacks and converts bool/int masks to f32 and back.
"""

import numpy as np

import concourse.bacc as bacc
import concourse.mybir as mybir
import concourse.tile as tile
from concourse.bass_utils import run_bass_kernel_spmd

# Problem constants (hardcoded per contract; kernel.py must be self-contained).
N_CORES = 8
C = 91
D = 256
N_TOTAL = 131072
N_SHARD = N_TOTAL // N_CORES  # 16384
P = 128
T = N_SHARD // P  # 128 tiles per core
MOMENTUM = 0.999
WARMUP_STEPS = 200
BASE_MOM = 0.99
SHADOW_M = min(MOMENTUM + 0.0009, 0.9999)  # 0.9999

# Tunables.
G = 8            # feature tiles per DMA group
F_BUFS = 4       # feature-tile buffering
ACT_ACCUM_EVERY = 5  # of every 5 tiles, 2 use the ACT square+accum path
ACT_ACCUM_COUNT = 2  # (a = 0.4); the rest use batched ACT square + DVE reduce

# Packed epilogue state layout (both input and output): [91, 516]
#   0:256   prototypes            -> new_proto
#   256:512 shadow_prototypes     -> new_shadow
#   512     proto_variance        -> new_var
#   513     proto_initialized     -> new_init
#   514     shadow_initialized    -> new_sinit
#   515     proto_update_count    -> new_count
W = 2 * D + 4  # 516

FP = mybir.dt.float32
FH = mybir.dt.float16
AF = mybir.ActivationFunctionType
OP = mybir.AluOpType

_cache: dict = {}


def _build(progress: float):
    nc = bacc.Bacc("TRN2", target_bir_lowering=False, debug=False,
                   num_devices=N_CORES)

    feat = nc.dram_tensor("feat", [N_SHARD, D], FP, kind="ExternalInput")
    labels_t = nc.dram_tensor("labels_t", [P, T], FP, kind="ExternalInput")
    epi_in = nc.dram_tensor("epi_in", [C, W], FP, kind="ExternalInput")
    out_all = nc.dram_tensor("out_all", [C, W], FP, kind="ExternalOutput")

    iota_const = nc.inline_tensor(
        np.tile(np.arange(C, dtype=np.float32), (P, 1)), name="iota_const")

    k_mom = float(np.float32(MOMENTUM - BASE_MOM))

    with tile.TileContext(nc) as tc:
        with (
            tc.tile_pool(name="const", bufs=1) as cpool,
            tc.tile_pool(name="feat", bufs=F_BUFS) as fpool,
            tc.tile_pool(name="oh", bufs=8) as ohpool,
            tc.tile_pool(name="stats", bufs=4) as spool,
            tc.tile_pool(name="scratch", bufs=2) as scrpool,
            tc.tile_pool(name="psum", bufs=1, space="PSUM") as pspool,
            tc.tile_pool(name="epi", bufs=1) as epool,
            tc.tile_pool(name="dram", bufs=1, space="DRAM") as dram,
        ):
            feat_r = feat[:, :].rearrange("(t p) d -> p t d", p=P)

            # First group's feature load goes first so DMA starts immediately.
            ftiles = []
            ftile0 = fpool.tile([P, G, D + 1], FH, tag="ftile", name="ftile0")
            nc.gpsimd.dma_start(ftile0[:, :, 0:D], feat_r[:, 0:G, :])
            ftiles.append(ftile0)

            # One-time constants / small inputs.
            iota_sb = cpool.tile([P, C], FP, tag="iota")
            nc.sync.dma_start(iota_sb[:], iota_const[:, :])
            labels_sb = cpool.tile([P, T], FP, tag="labels")
            nc.sync.dma_start(labels_sb[:], labels_t[:, :])
            epi_sb = epool.tile([C, W], FP, tag="epi_sb")
            nc.sync.dma_start(epi_sb[:], epi_in[:, :])

            proto_sb = epi_sb[:, 0:D]
            shadow_sb = epi_sb[:, D:2 * D]
            var_sb = epi_sb[:, 2 * D:2 * D + 1]
            init_sb = epi_sb[:, 2 * D + 1:2 * D + 2]
            sinit_sb = epi_sb[:, 2 * D + 2:2 * D + 3]
            count_sb = epi_sb[:, 2 * D + 3:2 * D + 4]

            psum = pspool.tile([C, D + 1], FP, tag="acc")

            n_groups = T // G
            for gi in range(n_groups):
                g0 = gi * G
                if gi == 0:
                    ftile = ftiles[0]
                else:
                    ftile = fpool.tile([P, G, D + 1], FH, tag="ftile",
                                       name=f"ftile{gi}")
                    nc.gpsimd.dma_start(ftile[:, :, 0:D],
                                        feat_r[:, g0:g0 + G, :])

                sumsq = spool.tile([P, G], FP, tag="sumsq")
                # split tiles between the two sumsq paths
                acc_tiles = [g for g in range(G)
                             if (g % ACT_ACCUM_EVERY) < ACT_ACCUM_COUNT]
                red_tiles = [g for g in range(G) if g not in acc_tiles]
                sq_scr = scrpool.tile([P, len(red_tiles), D], FH, tag="sq_scr")
                # batched ACT squares (no accumulator read)
                for j, g in enumerate(red_tiles):
                    nc.scalar.activation(
                        sq_scr[:, j, :], ftile[:, g, 0:D], AF.Square)
                # one batched DVE reduce for those tiles
                red_out = spool.tile([P, len(red_tiles)], FP, tag="red_out")
                nc.vector.tensor_reduce(
                    red_out[:], sq_scr[:], axis=mybir.AxisListType.X, op=OP.add)
                for j, g in enumerate(red_tiles):
                    pass  # red_out[:, j] corresponds to tile g
                # ACT square+accum path
                for g in acc_tiles:
                    nc.scalar.activation(
                        sq_scr[:, 0, :], ftile[:, g, 0:D], AF.Square,
                        accum_out=sumsq[:, g:g + 1])
                # gather red results into sumsq columns
                for j, g in enumerate(red_tiles):
                    nc.vector.tensor_copy(sumsq[:, g:g + 1],
                                          red_out[:, j:j + 1])

                norms = spool.tile([P, G], FP, tag="norms")
                nc.scalar.activation(norms[:], sumsq[:], AF.Sqrt)
                nc.vector.tensor_copy(ftile[:, :, D:D + 1], norms[:])
                rnorm = spool.tile([P, G], FP, tag="rnorm")
                nc.vector.reciprocal(rnorm[:], norms[:])

                for g in range(G):
                    t = g0 + g
                    oh = ohpool.tile([P, C], FH, tag="oh")
                    nc.vector.tensor_scalar(
                        oh[:], iota_sb[:],
                        labels_sb[:, t:t + 1], rnorm[:, g:g + 1],
                        op0=OP.is_equal, op1=OP.mult)
                    nc.tensor.matmul(
                        psum[:], oh[:], ftile[:, g, :],
                        start=(t == 0), stop=(t == T - 1))

            # ---- cross-core reduce of [C, 257] partials ----
            partial = epool.tile([C, D + 1], FP, tag="partial")
            nc.scalar.copy(partial[:], psum[:])
            cc_in = dram.tile([C, D + 1], FP, tag="cc_in")
            cc_out = dram.tile([C, D + 1], FP, tag="cc_out")
            nc.sync.dma_start(cc_in[:], partial[:])
            nc.gpsimd.collective_compute(
                "AllReduce", OP.add,
                replica_groups=[list(range(N_CORES))],
                ins=[cc_in[:].opt()], outs=[cc_out[:].opt()])
            total = epool.tile([C, D + 1], FP, tag="total")
            nc.sync.dma_start(total[:], cc_out[:])

            # ---- epilogue (identical on every core) ----
            sums = total[:, 0:D]
            counts_raw = total[:, D:D + 1]

            out_sb = epool.tile([C, W], FP, tag="out_sb")
            newp = out_sb[:, 0:D]
            news = out_sb[:, D:2 * D]
            newv = out_sb[:, 2 * D:2 * D + 1]
            newi = out_sb[:, 2 * D + 1:2 * D + 2]
            newsi = out_sb[:, 2 * D + 2:2 * D + 3]
            newc = out_sb[:, 2 * D + 3:2 * D + 4]

            def etile(tag, shape=(C, 1)):
                return epool.tile(list(shape), FP, tag=tag, name=tag)

            # round fp16-accumulated counts to exact ints (2^23 trick)
            counts = etile("counts")
            nc.vector.tensor_scalar(counts[:], counts_raw, float(2 ** 23),
                                    float(-(2 ** 23)), op0=OP.add, op1=OP.add)

            present = etile("present")
            nc.vector.tensor_scalar(present[:], counts[:], 0.0, None,
                                    op0=OP.is_gt)
            cntc = etile("cntc")
            nc.vector.tensor_scalar(cntc[:], counts[:], 1.0, None, op0=OP.max)
            inv = etile("inv")
            nc.vector.reciprocal(inv[:], cntc[:])
            cls = etile("cls", (C, D))
            nc.vector.tensor_scalar(cls[:], sums, inv[:], None, op0=OP.mult)

            # alpha = present * (1 - init*mom); mom = BASE + k*progress*e^-var
            e = etile("e")
            nc.scalar.activation(e[:], var_sb, AF.Exp, scale=-1.0)
            momt = etile("momt")
            nc.vector.tensor_scalar(momt[:], e[:], float(k_mom * progress),
                                    init_sb, op0=OP.mult, op1=OP.mult)
            a1 = etile("a1")
            nc.vector.tensor_scalar(a1[:], init_sb, float(-BASE_MOM), 1.0,
                                    op0=OP.mult, op1=OP.add)
            a2 = etile("a2")
            nc.vector.tensor_tensor(a2[:], a1[:], momt[:], op=OP.subtract)
            alpha = etile("alpha")
            nc.vector.tensor_tensor(alpha[:], a2[:], present[:], op=OP.mult)

            d = etile("d", (C, D))  # cls_feat - old
            nc.vector.tensor_tensor(d[:], cls[:], proto_sb, op=OP.subtract)
            t1 = etile("t1", (C, D))
            nc.vector.tensor_scalar(t1[:], d[:], alpha[:], None, op0=OP.mult)
            nc.vector.tensor_tensor(newp, proto_sb, t1[:], op=OP.add)

            sq91 = etile("sq91", (C, D))
            ss = etile("ss")
            nc.scalar.activation(sq91[:], d[:], AF.Square, accum_out=ss[:])
            mag = etile("mag")
            nc.scalar.activation(mag[:], ss[:], AF.Sqrt)

            w_m = etile("w_m")
            nc.vector.tensor_scalar(w_m[:], present[:], init_sb, 0.01,
                                    op0=OP.mult, op1=OP.mult)
            g2 = etile("g2")
            nc.vector.tensor_tensor(g2[:], mag[:], var_sb, op=OP.subtract)
            wg = etile("wg")
            nc.vector.tensor_tensor(wg[:], g2[:], w_m[:], op=OP.mult)
            nc.vector.tensor_tensor(newv, var_sb, wg[:], op=OP.add)

            b1 = etile("b1")
            nc.vector.tensor_scalar(b1[:], sinit_sb, float(-SHADOW_M), 1.0,
                                    op0=OP.mult, op1=OP.add)
            beta = etile("beta")
            nc.vector.tensor_tensor(beta[:], b1[:], present[:], op=OP.mult)
            d2 = etile("d2", (C, D))
            nc.vector.tensor_tensor(d2[:], newp, shadow_sb, op=OP.subtract)
            t2 = etile("t2", (C, D))
            nc.vector.tensor_scalar(t2[:], d2[:], beta[:], None, op0=OP.mult)
            nc.vector.tensor_tensor(news, shadow_sb, t2[:], op=OP.add)

            nc.vector.tensor_tensor(newi, init_sb, present[:], op=OP.max)
            nc.vector.tensor_tensor(newsi, sinit_sb, present[:], op=OP.max)
            nc.vector.tensor_tensor(newc, count_sb, present[:], op=OP.add)

            nc.sync.dma_start(out_all[:, :], out_sb[:])

    nc.finalize()
    return nc


def kernel(features, labels, prototypes, proto_initialized, proto_variance,
           shadow_prototypes, shadow_initialized, proto_update_count, step):
    features = np.ascontiguousarray(np.asarray(features, dtype=np.float32))
    labels = np.asarray(labels)
    prototypes = np.asarray(prototypes, dtype=np.float32)
    proto_initialized = np.asarray(proto_initialized)
    proto_variance = np.asarray(proto_variance, dtype=np.float32)
    shadow_prototypes = np.asarray(shadow_prototypes, dtype=np.float32)
    shadow_initialized = np.asarray(shadow_initialized)
    proto_update_count = np.asarray(proto_update_count)
    count_dtype = proto_update_count.dtype

    progress = min(1.0, float(step) / max(1, WARMUP_STEPS * 10))

    key = (features.shape, float(progress))
    nc = _cache.get(key)
    if nc is None:
        nc = _build(progress)
        _cache[key] = nc

    epi = np.empty((C, W), np.float32)
    epi[:, 0:D] = prototypes.reshape(C, D)
    epi[:, D:2 * D] = shadow_prototypes.reshape(C, D)
    epi[:, 2 * D] = proto_variance.reshape(C)
    epi[:, 2 * D + 1] = proto_initialized.reshape(C).astype(np.float32)
    epi[:, 2 * D + 2] = shadow_initialized.reshape(C).astype(np.float32)
    epi[:, 2 * D + 3] = proto_update_count.reshape(C).astype(np.float32)

    in_maps = []
    for i in range(N_CORES):
        sl = slice(i * N_SHARD, (i + 1) * N_SHARD)
        lab = labels[sl].astype(np.float32).reshape(T, P).T.copy()
        in_maps.append({
            "feat": features[sl],
            "labels_t": lab,
            "epi_in": epi,
        })

    res = run_bass_kernel_spmd(nc, in_maps, core_ids=list(range(N_CORES)))
    out = res.results[0]["out_all"]

    new_proto = out[:, 0:D].copy()
    new_shadow = out[:, D:2 * D].copy()
    new_var = out[:, 2 * D].copy()
    new_init = out[:, 2 * D + 1] > 0.5
    new_sinit = out[:, 2 * D + 2] > 0.5
    new_count = np.rint(out[:, 2 * D + 3]).astype(count_dtype)
    return (new_proto, new_var, new_shadow, new_init, new_sinit, new_count)
